# revision 3
# baseline (speedup 1.0000x reference)
"""Trainium2 Bass kernel for nn_Attention_7679401525457.

score_i = relu(Linear(tanh(concat(h_i, z)))); alphas = softmax(scores);
attention = sum_i alphas_i * h_i.

Data-parallel over 8 NeuronCores: batch dim (32) sharded 4-per-core; the
tiny W/b replicated. Each core reads its encoder slice from HBM exactly
once (SBUF-resident per batch) in 1 MiB chunks, computes scores with a
fused DVE multiply+reduce right behind the DMA stream, and streams the
weighted sum on the PE incrementally (softmax normalization is deferred
to the output row: alphas stay unnormalized because relu keeps scores in
[0, ~3], so exp never overflows).
"""

import numpy as np

import concourse.bass as bass
import concourse.bacc as bacc
import concourse.mybir as mybir
import concourse.tile as tile
from concourse.bass_utils import run_bass_kernel_spmd

B, S, D = 32, 1024, 1024
NCORES = 8
BPC = B // NCORES  # batches per core
NT = S // 128  # s-tiles per batch
CHUNK = 2  # s-tiles per DMA (2 tiles = 1 MiB)
NCH = NT // CHUNK
CW = CHUNK * D
F32 = mybir.dt.float32
F32R = mybir.dt.float32r
AF = mybir.ActivationFunctionType
ALU = mybir.AluOpType

# float32r: same bits as fp32, PE matmul runs 4x faster (TF32-like
# reduced mantissa in the array). Toggle if precision requires full fp32.
USE_F32R = True

_CACHE = {}


def _build(loop=None):
    import contextlib

    encdt = F32R if USE_F32R else F32
    nc = bacc.Bacc("TRN2", target_bir_lowering=False, debug=False)

    enc = nc.dram_tensor("enc", [BPC, S, D], F32, kind="ExternalInput")
    # zt[p, b*8+c] = z[b, p*8+c]   (z = decoder_hidden[-1] core slice)
    zt = nc.dram_tensor("zt", [128, BPC * 8], F32, kind="ExternalInput")
    w1rep = nc.dram_tensor("w1rep", [128, D], F32, kind="ExternalInput")
    # w2t[p, c] = W2[p*8+c]
    w2t = nc.dram_tensor("w2t", [128, 8], F32, kind="ExternalInput")
    # bb128 = b[0]/128 replicated, so a ones-matmul partition-sum adds b[0]
    bb128 = nc.dram_tensor("bb128", [128, 1], F32, kind="ExternalInput")
    att = nc.dram_tensor("att", [BPC, D], F32, kind="ExternalOutput")

    with tile.TileContext(nc) as tc:
        with (
            tc.tile_pool(name="const", bufs=1) as cpool,
            tc.tile_pool(name="encp", bufs=BPC) as encp,
            tc.tile_pool(name="ttp", bufs=2) as ttp,
            tc.tile_pool(name="junkp", bufs=2) as junkp,
            tc.tile_pool(name="smallp", bufs=4) as smallp,
            tc.tile_pool(name="orowp", bufs=2) as orowp,
            tc.tile_pool(name="pscb", bufs=1, space="PSUM") as pscb,
            tc.tile_pool(name="psp", bufs=2, space="PSUM") as psp,
        ):
            # ---- constants ----
            w1t = cpool.tile([128, D], F32)
            nc.sync.dma_start(w1t[:], w1rep.ap())
            ztt = cpool.tile([128, BPC * 8], F32)
            nc.sync.dma_start(ztt[:], zt.ap())
            w2tt = cpool.tile([128, 8], F32)
            nc.sync.dma_start(w2tt[:], w2t.ap())
            bbt = cpool.tile([128, 1], F32)
            nc.sync.dma_start(bbt[:], bb128.ap())
            ones128 = cpool.tile([128, 1], F32)
            nc.vector.memset(ones128[:], 1.0)
            ones_sq = cpool.tile([128, 128], F32)
            nc.vector.memset(ones_sq[:], 1.0)

            # ---- prepass: cb[:, b] = tanh(z_b) @ W2 + b0, on all partitions
            tz = cpool.tile([128, BPC * 8], F32)
            nc.scalar.activation(tz[:], ztt[:], AF.Tanh)
            czp = cpool.tile([128, BPC], F32)
            zjunk = cpool.tile([128, 8], F32)
            for bi in range(BPC):
                nc.vector.scalar_tensor_tensor(
                    out=zjunk[:],
                    in0=tz[:, bi * 8 : (bi + 1) * 8],
                    scalar=1.0,
                    in1=w2tt[:],
                    op0=ALU.mult,
                    op1=ALU.mult,
                    accum_out=czp[:, bi : bi + 1],
                )
            czp2 = cpool.tile([128, BPC], F32)
            nc.vector.tensor_scalar_add(czp2[:], czp[:], bbt[:, 0:1])
            cb_ps = pscb.tile([128, BPC], F32)
            nc.tensor.matmul(cb_ps[:], ones_sq[:], czp2[:], start=True, stop=True)
            cb = cpool.tile([128, BPC], F32)
            nc.scalar.copy(cb[:], cb_ps[:])

            # ---- per-batch pipeline, incremental in 1 MiB chunks ----
            lctx = tc.For_i(0, loop) if loop is not None else contextlib.nullcontext()
            with lctx:
              for bi in range(BPC):
                encT = encp.tile([128, NT * D], encdt, tag="enc")
                src = enc.ap()[bi].rearrange("(t p) d -> p t d", p=128)
                if USE_F32R:
                    src = src.bitcast(F32R)
                sc = smallp.tile([128, NT], F32, tag="sc")
                sr = smallp.tile([128, NT], F32, tag="sr")
                al = smallp.tile([128, NT], encdt, tag="al")
                ap0 = psp.tile([1, 512], F32, tag="ap0")
                ap1 = psp.tile([1, 512], F32, tag="ap1")
                for c in range(NCH):
                    nc.sync.dma_start(
                        encT[:, c * CW : (c + 1) * CW].rearrange(
                            "p (t d) -> p t d", t=CHUNK
                        ),
                        src[:, c * CHUNK : (c + 1) * CHUNK, :],
                    )
                    tt = ttp.tile([128, CW], F32, tag="tt")
                    tin = encT[:, c * CW : (c + 1) * CW]
                    if USE_F32R:
                        tin = tin.bitcast(F32)
                    nc.scalar.activation(tt[:], tin, AF.Tanh)
                    for k in range(CHUNK):
                        t = c * CHUNK + k
                        junk = junkp.tile([128, D], F32, tag="junk")
                        # fused multiply+row-sum: out=(tt*1)*w1, accum=Σ
                        # (tensor_tensor_reduce crashes the exec unit on
                        # this runtime; scalar_tensor_tensor accum works)
                        nc.vector.scalar_tensor_tensor(
                            out=junk[:],
                            in0=tt[:, k * D : (k + 1) * D],
                            scalar=1.0,
                            in1=w1t[:],
                            op0=ALU.mult,
                            op1=ALU.mult,
                            accum_out=sc[:, t : t + 1],
                        )
                    cols = slice(c * CHUNK, (c + 1) * CHUNK)
                    # relu(score + cb) in one DVE op, then exp on ACT.
                    # alphas stay unnormalized; normalization folds into
                    # the output row scale below.
                    nc.vector.tensor_scalar(
                        out=sr[:, cols],
                        in0=sc[:, cols],
                        scalar1=cb[:, bi : bi + 1],
                        scalar2=0.0,
                        op0=ALU.add,
                        op1=ALU.max,
                    )
                    nc.scalar.activation(al[:, cols], sr[:, cols], AF.Exp)
                    for k in range(CHUNK):
                        t = c * CHUNK + k
                        nc.tensor.matmul(
                            ap0[:],
                            al[:, t : t + 1],
                            encT[:, t * D : t * D + 512],
                            start=(t == 0),
                            stop=(t == NT - 1),
                        )
                        nc.tensor.matmul(
                            ap1[:],
                            al[:, t : t + 1],
                            encT[:, t * D + 512 : (t + 1) * D],
                            start=(t == 0),
                            stop=(t == NT - 1),
                        )

                # softmax denominator and output row
                rs = smallp.tile([128, 1], F32, tag="rs")
                al_f32 = al[:].bitcast(F32) if USE_F32R else al[:]
                nc.vector.tensor_reduce(
                    out=rs[:], in_=al_f32, axis=mybir.AxisListType.X, op=ALU.add
                )
                tot_ps = psp.tile([1, 1], F32, tag="tot")
                nc.tensor.matmul(tot_ps[:], ones128[:], rs[:], start=True, stop=True)
                recip = smallp.tile([1, 1], F32, tag="recip")
                nc.vector.reciprocal(recip[:], tot_ps[:])
                orow = orowp.tile([1, D], F32, tag="orow")
                nc.scalar.activation(
                    orow[:, 0:512], ap0[:], AF.Copy, scale=recip[0:1, 0:1]
                )
                nc.scalar.activation(
                    orow[:, 512:D], ap1[:], AF.Copy, scale=recip[0:1, 0:1]
                )
                nc.sync.dma_start(att.ap()[bi : bi + 1, :], orow[:])

    nc.compile()
    return nc


def _get_nc():
    if "nc" not in _CACHE:
        _CACHE["nc"] = _build()
    return _CACHE["nc"]


def _make_in_maps(encoder_outputs, decoder_hidden, W, b):
    enc = np.ascontiguousarray(np.asarray(encoder_outputs, dtype=np.float32))
    z = np.asarray(decoder_hidden, dtype=np.float32)[-1]  # [B, D]
    W = np.asarray(W, dtype=np.float32)
    b = np.asarray(b, dtype=np.float32)

    W1 = W[:D, 0]
    W2 = W[D:, 0]
    w1rep = np.ascontiguousarray(np.broadcast_to(W1[None, :], (128, D)))
    w2t = np.ascontiguousarray(W2.reshape(128, 8))
    bb128 = np.full((128, 1), float(b[0]) / 128.0, dtype=np.float32)

    in_maps = []
    for c in range(NCORES):
        zi = z[c * BPC : (c + 1) * BPC]  # [BPC, D]
        ztc = np.ascontiguousarray(
            zi.reshape(BPC, 128, 8).transpose(1, 0, 2).reshape(128, BPC * 8)
        )
        in_maps.append(
            {
                "enc": np.ascontiguousarray(enc[c * BPC : (c + 1) * BPC]),
                "zt": ztc,
                "w1rep": w1rep,
                "w2t": w2t,
                "bb128": bb128,
            }
        )
    return in_maps


def kernel(encoder_outputs, decoder_hidden, W, b, **_):
    in_maps = _make_in_maps(encoder_outputs, decoder_hidden, W, b)
    nc = _get_nc()
    res = run_bass_kernel_spmd(nc, in_maps, list(range(NCORES)))
    out = np.concatenate([res.results[c]["att"] for c in range(NCORES)], axis=0)
    return out.astype(np.float32)

